# revision 10
# baseline (speedup 1.0000x reference)
"""CrossAttLayer GNN message-passing kernel for TRN2 (8 NeuronCores).

v2: Q per-edge DMA gather eliminated.  Per core c owning nodes
[c*NPC, (c+1)*NPC):
  1. MLP tables on device (bf16): KV table for ALL nodes -> DRAM [TBL, 256]
     (row n = [k(n) | v(n)], natural h-major columns), Q table for local
     nodes -> DRAM [TBLQ, 128].
  2. Edges sorted by dst on host, grouped into NWIN windows of W=125 nodes,
     each padded to B blocks of 128 edges.  Per window:
       dma_gather KV rows by src (edge-major: edge e -> partition e%128,
       slot e//128); plain DMA of the window's 128 Q rows and of the
       host-built transposed one-hot AT[j, e] = (dst_in_win[e] == j).
       PE: qg[e,:] = AT_b^T @ Qw  (exact one-hot row select, 33 blocks)
       ACT: copy psum -> qg bf16
       DVE: A[e, j] one-hot via broadcast is_equal; P = k*qg;
       scores = reduce_fi(P); ACT exp(0.25 x); wv = v*exp (broadcast);
       PE: psum[128,136] += A_b^T @ [wv | exp]_b over B blocks
       epilogue: out[j] = psum_num[j] * recip(psum_den[j]) (broadcast)
  3. One DMA of [NPC, 128] f32 out; host concatenates the 8 core outputs.
"""

import numpy as np
import ml_dtypes

import sys
for _p in ("/opt/trn_rl_repo", "/root/.axon_site/_ro/trn_rl_repo"):
    if _p not in sys.path:
        sys.path.insert(0, _p)
import concourse.bass as bass
import concourse.bacc as bacc
import concourse.mybir as mybir
import concourse.tile as tile
from concourse.library_config import mlp as mlp_lib

BF16 = mybir.dt.bfloat16
FP8 = mybir.dt.float8e4
F32 = mybir.dt.float32
I16 = mybir.dt.int16
AF = mybir.ActivationFunctionType
ALU = mybir.AluOpType

H = 8          # heads
HD = 16        # head dim
D = 128        # feature dim


def make_meta(n_nodes, n_cores, win, nwin_per_core, B):
    npc = n_nodes // n_cores
    assert win * nwin_per_core == npc
    tbl = -(-n_nodes // 512) * 512       # KV table rows (mult of 512)
    tblq = -(-npc // 512) * 512          # Q table rows
    return dict(
        N=n_nodes, NC=n_cores, NPC=npc, W=win, NWIN=nwin_per_core, B=B,
        TBL=tbl, TBLQ=tblq, NQUEUES=4, QSPLIT=2,
    )


def build_nc(meta):
    PHASE = meta.get("PHASE", "all")
    REPS = int(meta.get("REPS", 1))
    NPC, W, NWIN, B = meta["NPC"], meta["W"], meta["NWIN"], meta["B"]
    TBL, TBLQ = meta["TBL"], meta["TBLQ"]
    NIDX = B * 128                      # gather idxs per window
    ICOL = NIDX // 16                   # idx columns per window
    KVF8 = bool(meta.get("KVF8"))
    KVDT = FP8 if KVF8 else BF16
    QF8 = bool(meta.get("QF8", meta.get("KVF8")))
    QDT = FP8 if QF8 else BF16

    nc = bacc.Bacc("TRN2", target_bir_lowering=False, debug=False,
                   num_devices=meta["NC"],
                   num_swdge_queues=int(meta.get("NQUEUES", 1)))

    # ---- I/O ----
    amT = nc.dram_tensor("amT", [128, TBL], BF16, kind="ExternalInput")
    hT = nc.dram_tensor("hT", [128, TBLQ], BF16, kind="ExternalInput")
    w1k = nc.dram_tensor("w1k", [128, 256], BF16, kind="ExternalInput")
    w1v = nc.dram_tensor("w1v", [128, 256], BF16, kind="ExternalInput")
    w1q = nc.dram_tensor("w1q", [128, 256], BF16, kind="ExternalInput")
    b1 = nc.dram_tensor("b1", [128, 6], F32, kind="ExternalInput")
    w2kv = nc.dram_tensor("w2kv", [128, 2, 256], BF16, kind="ExternalInput")
    w2q = nc.dram_tensor("w2q", [128, 2, 128], BF16, kind="ExternalInput")
    b2kv = nc.dram_tensor("b2kv", [1, 256], BF16, kind="ExternalInput")
    b2q = nc.dram_tensor("b2q", [1, 128], BF16, kind="ExternalInput")
    iota_in = nc.dram_tensor("iota", [128, 128], BF16, kind="ExternalInput")
    src_idx = nc.dram_tensor("src_idx", [128, NWIN * ICOL], I16,
                             kind="ExternalInput")
    at_in = nc.dram_tensor("at", [128, NWIN, NIDX], QDT,
                           kind="ExternalInput")
    dstw = nc.dram_tensor("dstw", [128, NWIN, B], F32, kind="ExternalInput")
    out_d = nc.dram_tensor("out", [NPC, D], F32, kind="ExternalOutput")
    if PHASE == "edge":
        kvtab_in = nc.dram_tensor("kvtab", [TBL, 256], KVDT, kind="ExternalInput")
        qtab_in = nc.dram_tensor("qtab", [TBLQ, D], QDT, kind="ExternalInput")

    with tile.TileContext(nc) as tc:
        nc.gpsimd.load_library(mlp_lib)
        with (
            tc.tile_pool(name="singles", bufs=1) as singles,
            tc.tile_pool(name="dram", bufs=1, space="DRAM") as dram,
        ):
            # ---- resident tiles ----
            w1k_sb = singles.tile([128, 256], BF16, tag="w1k")
            nc.sync.dma_start(w1k_sb[:], w1k[:])
            w1v_sb = singles.tile([128, 256], BF16, tag="w1v")
            nc.sync.dma_start(w1v_sb[:], w1v[:])
            w1q_sb = singles.tile([128, 256], BF16, tag="w1q")
            nc.sync.dma_start(w1q_sb[:], w1q[:])
            b1_sb = singles.tile([128, 6], F32, tag="b1")
            nc.sync.dma_start(b1_sb[:], b1[:])
            w2kv_sb = singles.tile([128, 2, 256], BF16, tag="w2kv")
            nc.sync.dma_start(w2kv_sb[:], w2kv[:])
            w2q_sb = singles.tile([128, 2, 128], BF16, tag="w2q")
            nc.sync.dma_start(w2q_sb[:], w2q[:])
            b2kv_sb = singles.tile([1, 256], BF16, tag="b2kv")
            nc.sync.dma_start(b2kv_sb[:], b2kv[:])
            b2q_sb = singles.tile([1, 128], BF16, tag="b2q")
            nc.sync.dma_start(b2q_sb[:], b2q[:])
            iota_sb = singles.tile([128, 128], BF16, tag="iota")
            nc.sync.dma_start(iota_sb[:], iota_in[:])
            srcidx_sb = singles.tile([128, NWIN * ICOL], I16, tag="srcidx")
            nc.sync.dma_start(srcidx_sb[:], src_idx[:])
            dstw_sb = singles.tile([128, NWIN, B], F32, tag="dstw")
            nc.sync.dma_start(dstw_sb[:], dstw[:])
            ones_sb = singles.tile([1, 128], BF16, tag="ones")
            nc.vector.memset(ones_sb[:], 1.0)
            out_sb = singles.tile([128, NWIN, D], F32, tag="outsb")

            if PHASE == "edge":
                kv_t, q_t = kvtab_in, qtab_in
            else:
                kv_t = dram.tile([TBL, 256], KVDT, tag="kvt")
                q_t = dram.tile([TBLQ, D], QDT, tag="qt")

            for rep in range(REPS):
                if PHASE != "edge":
                    with (
                        tc.tile_pool(name="mlpin", bufs=1) as mlpin,
                        tc.tile_pool(name="h1", bufs=2) as h1p,
                        tc.tile_pool(name="kvsb", bufs=2) as kvsbp,
                        tc.tile_pool(name="ps1", bufs=2, space="PSUM") as ps1p,
                        tc.tile_pool(name="ps2", bufs=2, space="PSUM") as ps2p,
                    ):
                        amT_sb = mlpin.tile([128, TBL], BF16, tag="amT")
                        nc.sync.dma_start(amT_sb[:], amT[:])
                        hT_sb = mlpin.tile([128, TBLQ], BF16, tag="hT")
                        nc.sync.dma_start(hT_sb[:], hT[:])

                        def relu_bias(dst, ps, bcol):
                            nc.scalar.activation(dst, ps, AF.Relu,
                                                 bias=b1_sb[:, bcol:bcol + 1])

                        # ---- MLP tables ----
                        # Q table first: unblocks Q loads during KV MLP
                        nchq = TBLQ // 512
                        for ci in range(nchq):
                            xs = hT_sb[:, ci * 512:(ci + 1) * 512]
                            h1q = h1p.tile([128, 2, 512], BF16, tag="h1q")
                            for hf in range(2):
                                ps = ps1p.tile([128, 512], F32, tag="ps1")
                                nc.tensor.matmul(ps[:], w1q_sb[:, hf * 128:(hf + 1) * 128],
                                                 xs, start=True, stop=True)
                                relu_bias(h1q[:, hf, :], ps[:], 4 + hf)
                            q_sb = kvsbp.tile([128, 4, 128], QDT, tag="qsb")
                            ps2 = ps2p.tile([128, 4, 128], F32, tag="ps2k")
                            for blk in range(4):
                                bs = blk * 128
                                for c2 in range(2):
                                    nc.tensor.matmul(ps2[:, blk, :],
                                                     h1q[:, c2, bs:bs + 128],
                                                     w2q_sb[:, c2, :],
                                                     start=(c2 == 0), stop=False)
                                nc.tensor.matmul(ps2[:, blk, :], ones_sb[:], b2q_sb[:],
                                                 start=False, stop=True)
                            nc.vector.tensor_copy(q_sb[:], ps2[:])
                            nc.sync.dma_start(
                                q_t[ci * 512:(ci + 1) * 512, :].rearrange(
                                    "(b p) f -> p b f", p=128),
                                q_sb[:])

                        # KV table: interleave k and v MLP per 512-row chunk
                        nch = TBL // 512
                        for ci in range(nch):
                            xs = amT_sb[:, ci * 512:(ci + 1) * 512]
                            h1k = h1p.tile([128, 2, 512], BF16, tag="h1k")
                            h1v = h1p.tile([128, 2, 512], BF16, tag="h1v")
                            for (h1t, w1_sb, b1c0) in ((h1k, w1k_sb, 0), (h1v, w1v_sb, 2)):
                                for hf in range(2):
                                    ps = ps1p.tile([128, 512], F32, tag="ps1")
                                    nc.tensor.matmul(ps[:],
                                                     w1_sb[:, hf * 128:(hf + 1) * 128],
                                                     xs, start=True, stop=True)
                                    relu_bias(h1t[:, hf, :], ps[:], b1c0 + hf)
                            kv_sb = kvsbp.tile([128, 4, 256], KVDT, tag="kvsb")
                            ps2k = ps2p.tile([128, 4, 128], F32, tag="ps2k")
                            ps2v = ps2p.tile([128, 4, 128], F32, tag="ps2v")
                            for blk in range(4):
                                bs = blk * 128
                                for c2 in range(2):
                                    nc.tensor.matmul(ps2k[:, blk, :],
                                                     h1k[:, c2, bs:bs + 128],
                                                     w2kv_sb[:, c2, 0:128],
                                                     start=(c2 == 0), stop=False)
                                nc.tensor.matmul(ps2k[:, blk, :], ones_sb[:],
                                                 b2kv_sb[:, 0:128],
                                                 start=False, stop=True)
                                for c2 in range(2):
                                    nc.tensor.matmul(ps2v[:, blk, :],
                                                     h1v[:, c2, bs:bs + 128],
                                                     w2kv_sb[:, c2, 128:256],
                                                     start=(c2 == 0), stop=False)
                                nc.tensor.matmul(ps2v[:, blk, :], ones_sb[:],
                                                 b2kv_sb[:, 128:256],
                                                 start=False, stop=True)
                            nc.vector.tensor_copy(kv_sb[:, :, 0:128], ps2k[:])
                            nc.scalar.activation(kv_sb[:, :, 128:256], ps2v[:],
                                                 AF.Copy)
                            nc.sync.dma_start(
                                kv_t[ci * 512:(ci + 1) * 512, :].rearrange(
                                    "(b p) f -> p b f", p=128),
                                kv_sb[:])

                if PHASE == "mlp":
                    nc.vector.memset(out_sb[:], 0.0)
                if PHASE != "mlp":
                    NOGATHER = bool(meta.get("NOGATHER"))
                    NOCOMPUTE = bool(meta.get("NOCOMPUTE"))
                    NSPLIT = int(meta.get("QSPLIT", 2))
                    NSUB = -(-B // 8)           # qg-expand psum sub-chunks
                    with (
                        tc.tile_pool(name="kvgp", bufs=int(meta.get("KVGB", 4))) as kvgp,
                        tc.tile_pool(name="atp", bufs=int(meta.get("ATB", 2))) as atp,
                        tc.tile_pool(name="qwp", bufs=2) as qwp,
                        tc.tile_pool(name="qgp", bufs=2) as qgp,
                        tc.tile_pool(name="edge", bufs=2) as edgep,
                        tc.tile_pool(name="wva", bufs=2) as wvap,
                        tc.tile_pool(name="psw", bufs=2, space="PSUM") as pswp,
                        tc.tile_pool(name="psq", bufs=2, space="PSUM") as psqp,
                    ):
                        if NOGATHER:
                            kvg_c = singles.tile([128, B, 256], KVDT, tag="kvgc")
                            nc.vector.memset(kvg_c[:], 0.25)
                            at_c = singles.tile([128, NIDX], QDT, tag="atc")
                            nc.vector.memset(at_c[:], 0.0)
                            qw_c = singles.tile([128, 128], QDT, tag="qwc")
                            nc.vector.memset(qw_c[:], 0.25)
                        if NOCOMPUTE:
                            nc.vector.memset(out_sb[:], 0.0)
                        # ---- edge phase ----
                        for w in range(NWIN):
                            ic = w * ICOL
                            if NOGATHER:
                                kvg, at_sb, qw_sb = kvg_c, at_c, qw_c
                            else:
                                kvg = kvgp.tile([128, B, 256], KVDT, tag="kvg")
                                # split KV gather into NSPLIT chunks; queue
                                # pairs alternate with window parity
                                bnds = [round(B * i / NSPLIT)
                                        for i in range(NSPLIT + 1)]
                                for s in range(NSPLIT):
                                    b0, b1_ = bnds[s], bnds[s + 1]
                                    if b0 == b1_:
                                        continue
                                    n = (b1_ - b0) * 128
                                    c0, cn = b0 * 8, (b1_ - b0) * 8
                                    nc.gpsimd.dma_gather(
                                        kvg[:, b0:b1_, :], kv_t[:],
                                        srcidx_sb[:, ic + c0:ic + c0 + cn],
                                        n, n, 256,
                                        single_packet=bool(meta.get("SP")),
                                        queue_num=(2 * (w % 2) + s) % 4)
                                at_sb = atp.tile([128, NIDX], QDT, tag="at")
                                nc.sync.dma_start(at_sb[:], at_in[:, w, :])
                                qw_sb = qwp.tile([128, 128], QDT, tag="qw")
                                nc.sync.dma_start(qw_sb[:],
                                                  q_t[w * W:w * W + 128, :])
                            if NOCOMPUTE:
                                continue

                            # qg[e, f] = Qw[dst_in_win[e], f] via one-hot matmul
                            qg = qgp.tile([128, B, 128], BF16, tag="qg")
                            for s in range(NSUB):
                                nb = min(8, B - s * 8)
                                psq = psqp.tile([128, 8, 128], F32, tag="psq")
                                for i in range(nb):
                                    blk = s * 8 + i
                                    nc.tensor.matmul(
                                        psq[:, i, :],
                                        at_sb[:, blk * 128:(blk + 1) * 128],
                                        qw_sb[:], start=True, stop=True)
                                nc.scalar.activation(qg[:, s * 8:s * 8 + nb, :],
                                                     psq[:, 0:nb, :], AF.Copy)

                            # A[e, j] one-hot (single broadcast is_equal)
                            A = wvap.tile([128, B, 128], BF16, tag="A")
                            nc.vector.tensor_tensor(
                                A[:],
                                iota_sb[:].unsqueeze(1).broadcast_to([128, B, 128]),
                                dstw_sb[:, w, :].unsqueeze(2).broadcast_to([128, B, 128]),
                                ALU.is_equal)

                            # P = k * qg ; scores = sum_fi P (per head)
                            P = edgep.tile([128, B, 128], BF16, tag="P")
                            nc.vector.tensor_tensor(P[:], kvg[:, :, 0:128], qg[:],
                                                    ALU.mult)
                            scores = edgep.tile([128, B, H], F32, tag="scores")
                            nc.vector.tensor_reduce(
                                scores[:],
                                P[:].rearrange("p s (h f) -> p s h f", f=HD),
                                mybir.AxisListType.X, ALU.add)
                            # exp(scores / 4) straight into wvx cols 128:136
                            wvx = wvap.tile([128, B, 136], BF16, tag="wvx")
                            nc.scalar.activation(wvx[:, :, 128:136], scores[:],
                                                 AF.Exp, scale=0.25)
                            nc.vector.tensor_tensor(
                                wvx[:, :, 0:128].rearrange(
                                    "p s (h f) -> p s h f", f=HD),
                                kvg[:, :, 128:256].rearrange(
                                    "p s (h f) -> p s h f", f=HD),
                                wvx[:, :, 128:136].unsqueeze(3).broadcast_to(
                                    [128, B, H, HD]),
                                ALU.mult)
                            # segment matmul
                            psw = pswp.tile([128, 136], F32, tag="psw")
                            for b in range(B):
                                nc.tensor.matmul(psw[:], A[:, b, :], wvx[:, b, :],
                                                 start=(b == 0), stop=(b == B - 1))
                            # epilogue
                            rec = edgep.tile([128, H], F32, tag="rec")
                            sden = edgep.tile([128, H], F32, tag="sden")
                            nc.vector.tensor_scalar_add(sden[:], psw[:, 128:136],
                                                        1e-20)
                            nc.vector.reciprocal(rec[:], sden[:])
                            nc.vector.tensor_tensor(
                                out_sb[0:W, w, :].rearrange(
                                    "p (h f) -> p h f", f=HD),
                                psw[0:W, 0:128].rearrange(
                                    "p (h f) -> p h f", f=HD),
                                rec[0:W, :].unsqueeze(2).broadcast_to([W, H, HD]),
                                ALU.mult)

            nc.sync.dma_start(
                out_d[:].rearrange("(w p) f -> p w f", p=W),
                out_sb[0:W, :, :])

    nc.compile()
    return nc


# ---------------- host side ----------------

def _wrap_idx(idx_flat):
    """[n] int16 -> [128, n/16] in dma_gather layout (i at [i%16, i//16],
    replicated across the 8 groups of 16 partitions)."""
    n = idx_flat.shape[0]
    assert n % 16 == 0
    w = idx_flat.reshape(-1, 16).T.astype(np.int16)
    return np.tile(w, (8, 1))


def host_prep(inputs, meta):
    """Build per-core in_maps. inputs: the full problem inputs (numpy)."""
    N, NC, NPC, W, NWIN, B = (meta["N"], meta["NC"], meta["NPC"], meta["W"],
                              meta["NWIN"], meta["B"])
    TBL, TBLQ = meta["TBL"], meta["TBLQ"]
    bf = ml_dtypes.bfloat16

    h = np.asarray(inputs["h"], np.float32)
    a_mol = np.asarray(inputs["a_mol"], np.float32)
    src = np.asarray(inputs["src"]).astype(np.int64)
    dst = np.asarray(inputs["dst"]).astype(np.int64)
    ci = np.asarray(inputs["center_index"]).astype(np.int64)

    inv = np.zeros(N, np.int64)
    inv[ci] = np.arange(N)
    dstn = inv[dst]

    order = np.argsort(dstn, kind="stable")
    src_s = src[order]
    dst_s = dstn[order]

    # shared tensors
    amT = np.zeros((128, TBL), bf)
    amT[:, :N] = a_mol.T.astype(bf)
    iota = np.tile(np.arange(128, dtype=np.float32).astype(bf), (128, 1))

    def l1w(w1):
        return np.asarray(w1, np.float32).astype(bf)          # [128, 256]

    w1k, w1v, w1q = l1w(inputs["Wk1"]), l1w(inputs["Wv1"]), l1w(inputs["Wq1"])
    b1 = np.stack([np.asarray(inputs[f"b{n}1"], np.float32).reshape(2, 128)[hf]
                   for n in ("k", "v", "q") for hf in range(2)], axis=1)
    b1 = np.ascontiguousarray(b1, np.float32)  # [128, 6]

    def l2w(w2):
        w2p = np.asarray(w2, np.float32).astype(bf)               # [256, 128]
        return w2p.reshape(2, 128, 128).transpose(1, 0, 2)        # [128, 2, 128]

    w2kp, w2vp, w2qp = l2w(inputs["Wk2"]), l2w(inputs["Wv2"]), l2w(inputs["Wq2"])
    w2kv = np.concatenate([w2kp, w2vp], axis=2)                   # [128, 2, 256]
    b2k = np.asarray(inputs["bk2"], np.float32).astype(bf)
    b2v = np.asarray(inputs["bv2"], np.float32).astype(bf)
    b2q = np.asarray(inputs["bq2"], np.float32).astype(bf)
    b2kv = np.concatenate([b2k, b2v]).reshape(1, 256)
    b2q = b2q.reshape(1, 128)

    shared = dict(amT=amT, w1k=w1k, w1v=w1v, w1q=w1q, b1=b1, w2kv=w2kv,
                  w2q=np.ascontiguousarray(w2qp), b2kv=b2kv, b2q=b2q,
                  iota=iota)

    in_maps = []
    for c in range(NC):
        lo, hi = c * NPC, (c + 1) * NPC
        hT = np.zeros((128, TBLQ), bf)
        hT[:, :NPC] = h[lo:hi].T.astype(bf)

        m = (dst_s >= lo) & (dst_s < hi)
        csrc = src_s[m]
        cdst = dst_s[m] - lo
        win = cdst // W
        # per-window padded edge lists
        NIDX = B * 128
        src_pad = np.zeros((NWIN, NIDX), np.int64)
        dw_pad = np.full((NWIN, NIDX), 255, np.int64)
        for w in range(NWIN):
            e = np.nonzero(win == w)[0]
            k = e.shape[0]
            assert k <= NIDX, (c, w, k, NIDX)
            src_pad[w, :k] = csrc[e]
            dw_pad[w, :k] = cdst[e] - w * W
        src_idx = np.concatenate([_wrap_idx(src_pad[w]) for w in range(NWIN)],
                                 axis=1)
        # transposed one-hot: at[j, w, i] = (dst_in_win == j)
        at_dt = (ml_dtypes.float8_e4m3 if meta.get("QF8", meta.get("KVF8"))
                 else bf)
        at = (dw_pad[None, :, :] == np.arange(128)[:, None, None]).astype(at_dt)
        # edge e of window w -> partition e%128, block col e//128
        dstw = np.ascontiguousarray(
            dw_pad.reshape(NWIN, B, 128).transpose(2, 0, 1)).astype(np.float32)

        in_maps.append(dict(shared, hT=hT, src_idx=src_idx, at=at, dstw=dstw))
    return in_maps


def unshard(results, meta):
    return np.concatenate([r["out"] for r in results], axis=0)


# ---------------- entry point ----------------

def default_meta(inputs):
    N, NC, W_, NWIN = 20000, 8, 125, 20
    dst = np.asarray(inputs["dst"]).astype(np.int64)
    ci = np.asarray(inputs["center_index"]).astype(np.int64)
    inv = np.zeros(N, np.int64)
    inv[ci] = np.arange(N)
    deg = np.bincount(inv[dst], minlength=N)
    B = int(-(-deg.reshape(N // W_, W_).sum(1).max() // 128))
    meta = make_meta(N, NC, W_, NWIN, B)
    meta["KVF8"] = 1
    meta["QSPLIT"] = 1
    meta["ATB"] = 6
    return meta


def kernel(**inputs):
    """Full-problem entry: takes unsharded inputs, returns [20000, 128] f32."""
    from concourse.bass_utils import run_bass_kernel_spmd

    meta = default_meta(inputs)
    nc = build_nc(meta)
    in_maps = host_prep(inputs, meta)
    res = run_bass_kernel_spmd(nc, in_maps, core_ids=list(range(meta["NC"])))
    return unshard(res.results, meta).astype(np.float32)
